# revision 1
# baseline (speedup 1.0000x reference)
"""MinGRU cell kernel for Trainium2 (8 NeuronCores, data-parallel over batch).

Computes, for x:[B,T,D], motion_mag:[B,T]:
    tau = 1 + softplus(alpha) * sigmoid(mw*mm + mb)        (per b,t)
    z   = sigmoid((x @ Wz^T + bz) / tau)                   (B,T,H)
    ht  = x @ Wh^T + bh                                    (B,T,H)
    h_t = (1-z_t)*h_{t-1} + z_t*ht_t   (scan over t, h_0=0)

Strategy:
  - Shard B=32 across 8 cores (4 per core). Weights replicated.
  - On-chip layout: h on partitions, t on the free dim, so the recurrence is
    a HW tensor_tensor_scan per [128h, 512t] tile, carried across t-tiles via
    initial=prev[:, -1:].
  - Projections: lhsT = W^T chunks (stationary), rhs = x^T chunks (moving),
    float32r (full PE rate, near-fp32 accuracy, fp32 PSUM accumulation).
  - tau: 1/tau computed on host, DMA-broadcast across partitions per block;
    folded in via one fused scalar_tensor_tensor: u = (zpre + bz) * invtau.
  - z = sigmoid(u), a = sigmoid(-u) = 1-z on ACT; b = (hpre + bh) * z on DVE.
  - Host pre-transposes x to [d, b*t] per core and un-transposes the output.
"""

import sys

import numpy as np

if "/opt/trn_rl_repo" not in sys.path:
    sys.path.insert(0, "/opt/trn_rl_repo")

B, T, D, H = 32, 2048, 512, 512
NCORES = 8
BL = B // NCORES            # batch per core = 4
TBLK = 1024                 # t-columns per block (2 psum banks)
MMN = 512                   # matmul free-dim (1 psum bank)
NTB = T // TBLK             # 2 t-blocks per sample
DC = D // 128               # 4 contraction chunks
HC = H // 128               # 4 h partition chunks
BT = BL * T                 # 8192 columns per core

_CACHE = {}


def _build_nc(bz0=None, bh0=None):
    import concourse.bass as bass
    import concourse.bacc as bacc
    import concourse.mybir as mybir
    import concourse.tile as tile
    from contextlib import ExitStack

    f32 = mybir.dt.float32
    f32r = mybir.dt.float32r
    AF = mybir.ActivationFunctionType
    OP = mybir.AluOpType

    nc = bacc.Bacc("TRN2", target_bir_lowering=False, debug=False)

    xt_ext = nc.declare_dram_parameter("xt", [DC, 128, BT], f32r, isOutput=False)
    wzt_ext = nc.declare_dram_parameter("wzt", [HC, 128, DC, 128], f32r, isOutput=False)
    wht_ext = nc.declare_dram_parameter("wht", [HC, 128, DC, 128], f32r, isOutput=False)
    bz_ext = nc.declare_dram_parameter("bz", [HC, 128, 1], f32, isOutput=False)
    bh_ext = nc.declare_dram_parameter("bh", [HC, 128, 1], f32, isOutput=False)
    itau_ext = nc.declare_dram_parameter("invtau", [BL, 1, T], f32, isOutput=False)
    out_ext = nc.declare_dram_parameter("out", [BL, HC, 128, T], f32, isOutput=True)

    with tile.TileContext(nc) as tc, ExitStack() as ctx:
        singles = ctx.enter_context(tc.tile_pool(name="singles", bufs=1))
        x_pool = ctx.enter_context(tc.tile_pool(name="x", bufs=3))
        j_pool = ctx.enter_context(tc.tile_pool(name="j", bufs=3))
        psum = ctx.enter_context(tc.tile_pool(name="psum", bufs=2, space="PSUM"))
        work = ctx.enter_context(tc.tile_pool(name="work", bufs=4))
        ab_pool = ctx.enter_context(tc.tile_pool(name="ab", bufs=4))
        h_pool = ctx.enter_context(tc.tile_pool(name="h", bufs=8))

        # Weights are hc-major in DRAM: the first matmul group (hc=0) only
        # needs a 256KB DMA. First block's x arrives as 512-col halves so the
        # first 8-matmul group is gated on ~1.3MB instead of 3MB.
        wz_hc, wh_hc = [None] * HC, [None] * HC
        xs0h = [[None] * DC for _ in range(2)]
        wz_hc[0] = singles.tile([128, DC * 128], f32r, tag="wzhc0", name="wzhc0")
        nc.sync.dma_start(out=wz_hc[0][:], in_=wzt_ext[0])
        for dc in range(DC):
            xt = x_pool.tile([128, MMN], f32r, tag=f"x{dc}", name=f"x0a_{dc}")
            nc.sync.dma_start(out=xt[:], in_=xt_ext[dc, :, 0:MMN])
            xs0h[0][dc] = xt
        wh_hc[0] = singles.tile([128, DC * 128], f32r, tag="whhc0", name="whhc0")
        nc.sync.dma_start(out=wh_hc[0][:], in_=wht_ext[0])
        for dc in range(DC):
            xt = x_pool.tile([128, MMN], f32r, tag=f"x{dc}", name=f"x0b_{dc}")
            nc.sync.dma_start(out=xt[:], in_=xt_ext[dc, :, MMN:TBLK])
            xs0h[1][dc] = xt
        for hc in range(1, HC):
            w = singles.tile([128, DC * 128], f32r, tag=f"wzhc{hc}", name=f"wzhc{hc}")
            nc.sync.dma_start(out=w[:], in_=wzt_ext[hc])
            wz_hc[hc] = w
            w = singles.tile([128, DC * 128], f32r, tag=f"whhc{hc}", name=f"whhc{hc}")
            nc.sync.dma_start(out=w[:], in_=wht_ext[hc])
            wh_hc[hc] = w
        # gpsimd queue: the first block's 1/tau halves go first; bias columns
        # are DMA'd only when non-uniform (uniform biases ride as immediates).
        jt0 = j_pool.tile([128, TBLK], f32, tag="J", name="jt0")
        for half in range(2):
            iv0 = itau_ext[0, 0, half * MMN:(half + 1) * MMN]
            iv0_b = bass.AP(
                tensor=iv0.tensor, offset=iv0.offset, ap=[[0, 128]] + list(iv0.ap)
            )
            nc.gpsimd.dma_start(out=jt0[:, half * MMN:(half + 1) * MMN], in_=iv0_b)
        bz_col = [bz0] * HC
        bh_col = [bh0] * HC
        if bz0 is None:
            bz_col = []
            for hc in range(HC):
                bzc = singles.tile([128, 1], f32, tag=f"bz{hc}", name=f"bzc{hc}")
                nc.gpsimd.dma_start(out=bzc[:], in_=bz_ext[hc])
                bz_col.append(bzc[:])
        if bh0 is None:
            bh_col = []
            for hc in range(HC):
                bhc = singles.tile([128, 1], f32, tag=f"bh{hc}", name=f"bhc{hc}")
                nc.gpsimd.dma_start(out=bhc[:], in_=bh_ext[hc])
                bh_col.append(bhc[:])

        h_prev = [[None] * HC for _ in range(BL)]

        for b in range(BL):
            for tb in range(NTB):
                bt0 = b * T + tb * TBLK
                ts = slice(tb * TBLK, (tb + 1) * TBLK)
                first_blk = (b == 0 and tb == 0)
                if first_blk:
                    xs = None
                else:
                    xs = []
                    for dc in range(DC):
                        xt = x_pool.tile([128, TBLK], f32r, tag=f"x{dc}")
                        nc.sync.dma_start(
                            out=xt[:], in_=xt_ext[dc, :, bt0:bt0 + TBLK]
                        )
                        xs.append(xt)
                # Broadcast 1/tau row across all 128 partitions.
                if b == 0 and tb == 0:
                    jt = jt0
                else:
                    jt = j_pool.tile([128, TBLK], f32, tag="J")
                    iv = itau_ext[b, 0, tb * TBLK:(tb + 1) * TBLK]
                    iv_b = bass.AP(
                        tensor=iv.tensor, offset=iv.offset,
                        ap=[[0, 128]] + list(iv.ap),
                    )
                    nc.gpsimd.dma_start(out=jt[:], in_=iv_b)

                for hc in range(HC):
                    # First block's hc0 and the very last chain run per
                    # 512-col half: shorter pipeline ramp in, and the final
                    # out-DMA halves and overlaps the last half-scan.
                    last_blk = (b == BL - 1 and tb == NTB - 1 and hc == HC - 1)
                    nsub = 2 if ((first_blk and hc == 0) or last_blk) else 1
                    width = TBLK // nsub

                    u = work.tile([128, TBLK], f32, tag="u")
                    z = work.tile([128, TBLK], f32, tag="z")
                    a = ab_pool.tile([128, TBLK], f32, tag="a")
                    bb = ab_pool.tile([128, TBLK], f32, tag="b")
                    h = h_pool.tile([128, TBLK], f32, tag="h")

                    for sub in range(nsub):
                        ssl = slice(sub * width, (sub + 1) * width)
                        zq = psum.tile([128, width], f32, tag="zq")
                        hq = psum.tile([128, width], f32, tag="hq")
                        for half in range(width // MMN):
                            h0 = sub * width // MMN + half
                            csl = slice(h0 * MMN, (h0 + 1) * MMN)
                            psl = slice(half * MMN, (half + 1) * MMN)
                            for dc in range(DC):
                                rhs = (xs0h[h0][dc][:] if first_blk
                                       else xs[dc][:, csl])
                                nc.tensor.matmul(
                                    zq[:, psl],
                                    lhsT=wz_hc[hc][:, dc * 128:(dc + 1) * 128],
                                    rhs=rhs,
                                    start=(dc == 0),
                                    stop=(dc == DC - 1),
                                )
                        for half in range(width // MMN):
                            h0 = sub * width // MMN + half
                            csl = slice(h0 * MMN, (h0 + 1) * MMN)
                            psl = slice(half * MMN, (half + 1) * MMN)
                            for dc in range(DC):
                                rhs = (xs0h[h0][dc][:] if first_blk
                                       else xs[dc][:, csl])
                                nc.tensor.matmul(
                                    hq[:, psl],
                                    lhsT=wh_hc[hc][:, dc * 128:(dc + 1) * 128],
                                    rhs=rhs,
                                    start=(dc == 0),
                                    stop=(dc == DC - 1),
                                )

                        # u = (zpre + bz) * invtau
                        nc.vector.scalar_tensor_tensor(
                            u[:, ssl], zq[:], bz_col[hc], jt[:, ssl],
                            op0=OP.add, op1=OP.mult,
                        )
                        nc.scalar.activation(z[:, ssl], u[:, ssl], AF.Sigmoid)
                        nc.scalar.activation(
                            a[:, ssl], u[:, ssl], AF.Sigmoid, scale=-1.0
                        )
                        # b = (hpre + bh) * z
                        nc.vector.scalar_tensor_tensor(
                            bb[:, ssl], hq[:], bh_col[hc], z[:, ssl],
                            op0=OP.add, op1=OP.mult,
                        )
                        init = (
                            (0.0 if tb == 0 else h_prev[b][hc][:, TBLK - 1:TBLK])
                            if sub == 0 else h[:, sub * width - 1:sub * width]
                        )
                        nc.vector.tensor_tensor_scan(
                            h[:, ssl], a[:, ssl], bb[:, ssl], init,
                            op0=OP.mult, op1=OP.add,
                        )
                        if nsub > 1:
                            osl = slice(tb * TBLK + sub * width,
                                        tb * TBLK + (sub + 1) * width)
                            nc.sync.dma_start(
                                out=out_ext[b, hc, :, osl], in_=h[:, ssl]
                            )
                    h_prev[b][hc] = h
                    if nsub == 1:
                        nc.sync.dma_start(out=out_ext[b, hc, :, ts], in_=h[:])

    nc.compile()
    return nc


def _prep_inputs(x, motion_mag, Wz, bz, Wh, bh, motion_weight, motion_bias, alpha):
    x = np.ascontiguousarray(np.asarray(x, dtype=np.float32))
    mm = np.asarray(motion_mag, dtype=np.float32)
    Wz = np.asarray(Wz, dtype=np.float32)
    Wh = np.asarray(Wh, dtype=np.float32)
    bz = np.asarray(bz, dtype=np.float32).reshape(HC, 128, 1)
    bh = np.asarray(bh, dtype=np.float32).reshape(HC, 128, 1)
    mw = float(np.asarray(motion_weight))
    mb = float(np.asarray(motion_bias))
    al = float(np.asarray(alpha))

    a_sp = float(np.log1p(np.exp(al)))  # softplus(alpha)
    sig = 1.0 / (1.0 + np.exp(-(mw * mm + mb)))
    invtau = (1.0 / (1.0 + a_sp * sig)).astype(np.float32)

    wzt = np.ascontiguousarray(
        Wz.T.reshape(DC, 128, HC, 128).transpose(2, 1, 0, 3))
    wht = np.ascontiguousarray(
        Wh.T.reshape(DC, 128, HC, 128).transpose(2, 1, 0, 3))

    in_maps = []
    for c in range(NCORES):
        xl = x[c * BL:(c + 1) * BL].reshape(BL * T, D)
        xt = np.ascontiguousarray(xl.T).reshape(DC, 128, BT)
        in_maps.append({
            "xt": xt,
            "wzt": wzt,
            "wht": wht,
            "bz": bz,
            "bh": bh,
            "invtau": np.ascontiguousarray(
                invtau[c * BL:(c + 1) * BL]).reshape(BL, 1, T),
        })
    return in_maps


def _assemble(results):
    outs = []
    for c in range(NCORES):
        o = results[c]["out"]  # [BL, HC, 128, T]
        o = np.transpose(o, (0, 3, 1, 2)).reshape(BL, T, H)
        outs.append(o)
    return np.ascontiguousarray(np.concatenate(outs, axis=0))


def _run(inputs, trace=False):
    from concourse.bass_utils import run_bass_kernel_spmd

    bza = np.asarray(inputs["bz"], dtype=np.float32).reshape(-1)
    bha = np.asarray(inputs["bh"], dtype=np.float32).reshape(-1)
    bz0 = float(bza[0]) if np.all(bza == bza[0]) else None
    bh0 = float(bha[0]) if np.all(bha == bha[0]) else None
    key = ("nc", bz0, bh0)
    if key not in _CACHE:
        _CACHE[key] = _build_nc(bz0, bh0)
    nc = _CACHE[key]
    in_maps = _prep_inputs(**inputs)
    res = run_bass_kernel_spmd(nc, in_maps, list(range(NCORES)), trace=trace)
    return _assemble(res.results), res


def kernel(**inputs):
    out, _ = _run(inputs, trace=False)
    return out



# revision 2
# speedup vs baseline: 1.0699x; 1.0699x over previous
"""MinGRU cell kernel for Trainium2 (8 NeuronCores, data-parallel over batch).

Computes, for x:[B,T,D], motion_mag:[B,T]:
    tau = 1 + softplus(alpha) * sigmoid(mw*mm + mb)        (per b,t)
    z   = sigmoid((x @ Wz^T + bz) / tau)                   (B,T,H)
    ht  = x @ Wh^T + bh                                    (B,T,H)
    h_t = (1-z_t)*h_{t-1} + z_t*ht_t   (scan over t, h_0=0)

Strategy:
  - Shard B=32 across 8 cores (4 per core). Weights replicated.
  - On-chip layout: h on partitions, t on the free dim, so the recurrence is
    a HW tensor_tensor_scan per [128h, 1024t] tile, carried across t-tiles via
    initial=prev[:, -1:].
  - Projections: lhsT = W^T chunks (stationary), rhs = x^T chunks (moving),
    float32r (full PE rate, near-fp32 accuracy, fp32 PSUM accumulation).
  - Post-GEMM pipeline balanced across engines (DVE was the bottleneck):
      DVE STT : u = (zq + bz) * invtau          (PSUM read, fp32 1x)
      ACT     : z = sigmoid(u)                  (bf16)
      DVE TS  : a = 1 - z                       (bf16, 4x mode)
      ACT     : ht = hq + bh  (PSUM->SBUF)      (bf16 out)
      DVE TT  : b = z * ht                      (bf16, 2x mode)
      DVE scan: h = scan(a, b)                  (bf16 io, fp32 state)
    Everything bf16 downstream of PSUM: DVE 16-bit modes double throughput
    and the output DMA halves. Host casts the bf16 output back to fp32.
  - A few dummy fp32 matmuls at t=0 warm the PE HAM clock-gate (2.4 GHz)
    while the first weight/x DMAs land.
  - tau: 1/tau computed on host (bf16), DMA-broadcast across partitions.
  - Host pre-transposes x to [d, b*t] per core and un-transposes the output.
"""

import sys

import numpy as np

if "/opt/trn_rl_repo" not in sys.path:
    sys.path.insert(0, "/opt/trn_rl_repo")

B, T, D, H = 32, 2048, 512, 512
NCORES = 8
BL = B // NCORES            # batch per core = 4
TBLK = 1024                 # t-columns per block (2 psum banks)
MMN = 512                   # matmul free-dim (1 psum bank)
NTB = T // TBLK             # 2 t-blocks per sample
DC = D // 128               # 4 contraction chunks
HC = H // 128               # 4 h partition chunks
BT = BL * T                 # 8192 columns per core

_CACHE = {}


def _build_nc(bz0=None, bh0=None):
    import concourse.bass as bass
    import concourse.bacc as bacc
    import concourse.mybir as mybir
    import concourse.tile as tile
    from contextlib import ExitStack

    f32 = mybir.dt.float32
    f32r = mybir.dt.float32r
    bf16 = mybir.dt.bfloat16
    AF = mybir.ActivationFunctionType
    OP = mybir.AluOpType

    nc = bacc.Bacc("TRN2", target_bir_lowering=False, debug=False)

    xt_ext = nc.declare_dram_parameter("xt", [DC, 128, BT], f32r, isOutput=False)
    wzt_ext = nc.declare_dram_parameter("wzt", [HC, 128, DC, 128], f32r, isOutput=False)
    wht_ext = nc.declare_dram_parameter("wht", [HC, 128, DC, 128], f32r, isOutput=False)
    bz_ext = nc.declare_dram_parameter("bz", [HC, 128, 1], f32, isOutput=False)
    bh_ext = nc.declare_dram_parameter("bh", [HC, 128, 1], f32, isOutput=False)
    itau_ext = nc.declare_dram_parameter("invtau", [BL, 1, T], bf16, isOutput=False)
    out_ext = nc.declare_dram_parameter("out", [BL, HC, 128, T], bf16, isOutput=True)

    with tile.TileContext(nc) as tc, ExitStack() as ctx:
        singles = ctx.enter_context(tc.tile_pool(name="singles", bufs=1))
        x_pool = ctx.enter_context(tc.tile_pool(name="x", bufs=3))
        j_pool = ctx.enter_context(tc.tile_pool(name="j", bufs=3))
        psum = ctx.enter_context(tc.tile_pool(name="psum", bufs=2, space="PSUM"))
        work = ctx.enter_context(tc.tile_pool(name="work", bufs=4))
        ab_pool = ctx.enter_context(tc.tile_pool(name="ab", bufs=4))
        h_pool = ctx.enter_context(tc.tile_pool(name="h", bufs=8))

        # HAM warm-up: a few dependency-free fp32 matmuls (1 col / 4 cycles,
        # so each is long) keep the PE busy while the first weight/x DMAs
        # land, flipping the clock-gate to 8/8 before the real GEMMs start.
        warm = singles.tile([128, MMN], f32, tag="warm", name="warm")
        nc.vector.memset(warm[:], 0.0)
        wq0 = psum.tile([128, MMN], f32, tag="zq", name="warmq")
        for i in range(3):
            nc.tensor.matmul(
                wq0[:], lhsT=warm[:, 0:128], rhs=warm[:], start=True, stop=True
            )

        # Weights are hc-major in DRAM: the first matmul group (hc=0) only
        # needs a 256KB DMA. First block's x arrives as 512-col halves so the
        # first 8-matmul group is gated on ~1.3MB instead of 3MB.
        wz_hc, wh_hc = [None] * HC, [None] * HC
        xs0h = [[None] * DC for _ in range(2)]
        wz_hc[0] = singles.tile([128, DC * 128], f32r, tag="wzhc0", name="wzhc0")
        nc.sync.dma_start(out=wz_hc[0][:], in_=wzt_ext[0])
        for dc in range(DC):
            xt = x_pool.tile([128, MMN], f32r, tag=f"x{dc}", name=f"x0a_{dc}")
            nc.sync.dma_start(out=xt[:], in_=xt_ext[dc, :, 0:MMN])
            xs0h[0][dc] = xt
        wh_hc[0] = singles.tile([128, DC * 128], f32r, tag="whhc0", name="whhc0")
        nc.sync.dma_start(out=wh_hc[0][:], in_=wht_ext[0])
        for dc in range(DC):
            xt = x_pool.tile([128, MMN], f32r, tag=f"x{dc}", name=f"x0b_{dc}")
            nc.sync.dma_start(out=xt[:], in_=xt_ext[dc, :, MMN:TBLK])
            xs0h[1][dc] = xt
        for hc in range(1, HC):
            w = singles.tile([128, DC * 128], f32r, tag=f"wzhc{hc}", name=f"wzhc{hc}")
            nc.sync.dma_start(out=w[:], in_=wzt_ext[hc])
            wz_hc[hc] = w
            w = singles.tile([128, DC * 128], f32r, tag=f"whhc{hc}", name=f"whhc{hc}")
            nc.sync.dma_start(out=w[:], in_=wht_ext[hc])
            wh_hc[hc] = w
        # gpsimd queue: the first block's 1/tau halves go first; bias columns
        # are DMA'd only when non-uniform (uniform biases ride as immediates).
        jt0 = j_pool.tile([128, TBLK], bf16, tag="J", name="jt0")
        for half in range(2):
            iv0 = itau_ext[0, 0, half * MMN:(half + 1) * MMN]
            iv0_b = bass.AP(
                tensor=iv0.tensor, offset=iv0.offset, ap=[[0, 128]] + list(iv0.ap)
            )
            nc.gpsimd.dma_start(out=jt0[:, half * MMN:(half + 1) * MMN], in_=iv0_b)
        bz_col = [bz0] * HC
        bh_col = [bh0] * HC
        if bz0 is None:
            bz_col = []
            for hc in range(HC):
                bzc = singles.tile([128, 1], f32, tag=f"bz{hc}", name=f"bzc{hc}")
                nc.gpsimd.dma_start(out=bzc[:], in_=bz_ext[hc])
                bz_col.append(bzc[:])
        if bh0 is None:
            bh_col = []
            for hc in range(HC):
                bhc = singles.tile([128, 1], f32, tag=f"bh{hc}", name=f"bhc{hc}")
                nc.gpsimd.dma_start(out=bhc[:], in_=bh_ext[hc])
                bh_col.append(bhc[:])

        h_prev = [[None] * HC for _ in range(BL)]

        for b in range(BL):
            for tb in range(NTB):
                bt0 = b * T + tb * TBLK
                ts = slice(tb * TBLK, (tb + 1) * TBLK)
                first_blk = (b == 0 and tb == 0)
                if first_blk:
                    xs = None
                else:
                    xs = []
                    for dc in range(DC):
                        xt = x_pool.tile([128, TBLK], f32r, tag=f"x{dc}")
                        nc.sync.dma_start(
                            out=xt[:], in_=xt_ext[dc, :, bt0:bt0 + TBLK]
                        )
                        xs.append(xt)
                # Broadcast 1/tau row across all 128 partitions.
                if b == 0 and tb == 0:
                    jt = jt0
                else:
                    jt = j_pool.tile([128, TBLK], bf16, tag="J")
                    iv = itau_ext[b, 0, tb * TBLK:(tb + 1) * TBLK]
                    iv_b = bass.AP(
                        tensor=iv.tensor, offset=iv.offset,
                        ap=[[0, 128]] + list(iv.ap),
                    )
                    nc.gpsimd.dma_start(out=jt[:], in_=iv_b)

                for hc in range(HC):
                    # First block's hc0 and the very last chain run per
                    # 512-col half: shorter pipeline ramp in, and the final
                    # out-DMA halves and overlaps the last half-scan.
                    last_blk = (b == BL - 1 and tb == NTB - 1 and hc == HC - 1)
                    nsub = 2 if ((first_blk and hc == 0) or last_blk) else 1
                    width = TBLK // nsub

                    u = work.tile([128, TBLK], bf16, tag="u")
                    z = work.tile([128, TBLK], bf16, tag="z")
                    a = ab_pool.tile([128, TBLK], bf16, tag="a")
                    bb = ab_pool.tile([128, TBLK], bf16, tag="b")
                    ht = ab_pool.tile([128, TBLK], bf16, tag="ht")
                    h = h_pool.tile([128, TBLK], bf16, tag="h")

                    for sub in range(nsub):
                        ssl = slice(sub * width, (sub + 1) * width)
                        zq = psum.tile([128, width], f32, tag="zq")
                        hq = psum.tile([128, width], f32, tag="hq")
                        for half in range(width // MMN):
                            h0 = sub * width // MMN + half
                            csl = slice(h0 * MMN, (h0 + 1) * MMN)
                            psl = slice(half * MMN, (half + 1) * MMN)
                            for dc in range(DC):
                                rhs = (xs0h[h0][dc][:] if first_blk
                                       else xs[dc][:, csl])
                                nc.tensor.matmul(
                                    zq[:, psl],
                                    lhsT=wz_hc[hc][:, dc * 128:(dc + 1) * 128],
                                    rhs=rhs,
                                    start=(dc == 0),
                                    stop=(dc == DC - 1),
                                )
                        for half in range(width // MMN):
                            h0 = sub * width // MMN + half
                            csl = slice(h0 * MMN, (h0 + 1) * MMN)
                            psl = slice(half * MMN, (half + 1) * MMN)
                            for dc in range(DC):
                                rhs = (xs0h[h0][dc][:] if first_blk
                                       else xs[dc][:, csl])
                                nc.tensor.matmul(
                                    hq[:, psl],
                                    lhsT=wh_hc[hc][:, dc * 128:(dc + 1) * 128],
                                    rhs=rhs,
                                    start=(dc == 0),
                                    stop=(dc == DC - 1),
                                )

                        # u = (zpre + bz) * invtau   [DVE, PSUM read]
                        nc.vector.scalar_tensor_tensor(
                            u[:, ssl], zq[:], bz_col[hc], jt[:, ssl],
                            op0=OP.add, op1=OP.mult,
                        )
                        # z = sigmoid(u)             [ACT]
                        nc.scalar.activation(z[:, ssl], u[:, ssl], AF.Sigmoid)
                        # a = 1 - z                  [DVE, bf16 4x]
                        nc.vector.tensor_scalar(
                            a[:, ssl], z[:, ssl], -1.0, 1.0,
                            op0=OP.mult, op1=OP.add,
                        )
                        # ht = hpre + bh  (PSUM evacuation)  [ACT]
                        if isinstance(bh_col[hc], float):
                            if bh_col[hc] == 0.0:
                                nc.scalar.activation(ht[:, ssl], hq[:], AF.Copy)
                            else:
                                nc.scalar.activation(
                                    ht[:, ssl], hq[:], AF.Identity,
                                    bias=bh_col[hc],
                                )
                        else:
                            nc.scalar.activation(
                                ht[:, ssl], hq[:], AF.Identity, bias=bh_col[hc],
                            )
                        # b = z * ht                 [DVE, bf16 2x]
                        nc.vector.tensor_tensor(
                            bb[:, ssl], z[:, ssl], ht[:, ssl], OP.mult
                        )
                        init = (
                            (0.0 if tb == 0 else h_prev[b][hc][:, TBLK - 1:TBLK])
                            if sub == 0 else h[:, sub * width - 1:sub * width]
                        )
                        nc.vector.tensor_tensor_scan(
                            h[:, ssl], a[:, ssl], bb[:, ssl], init,
                            op0=OP.mult, op1=OP.add,
                        )
                        if nsub > 1:
                            osl = slice(tb * TBLK + sub * width,
                                        tb * TBLK + (sub + 1) * width)
                            nc.sync.dma_start(
                                out=out_ext[b, hc, :, osl], in_=h[:, ssl]
                            )
                    h_prev[b][hc] = h
                    if nsub == 1:
                        nc.sync.dma_start(out=out_ext[b, hc, :, ts], in_=h[:])

    nc.compile()
    return nc


def _prep_inputs(x, motion_mag, Wz, bz, Wh, bh, motion_weight, motion_bias, alpha):
    import ml_dtypes

    x = np.ascontiguousarray(np.asarray(x, dtype=np.float32))
    mm = np.asarray(motion_mag, dtype=np.float32)
    Wz = np.asarray(Wz, dtype=np.float32)
    Wh = np.asarray(Wh, dtype=np.float32)
    bz = np.asarray(bz, dtype=np.float32).reshape(HC, 128, 1)
    bh = np.asarray(bh, dtype=np.float32).reshape(HC, 128, 1)
    mw = float(np.asarray(motion_weight))
    mb = float(np.asarray(motion_bias))
    al = float(np.asarray(alpha))

    a_sp = float(np.log1p(np.exp(al)))  # softplus(alpha)
    sig = 1.0 / (1.0 + np.exp(-(mw * mm + mb)))
    invtau = (1.0 / (1.0 + a_sp * sig)).astype(ml_dtypes.bfloat16)

    wzt = np.ascontiguousarray(
        Wz.T.reshape(DC, 128, HC, 128).transpose(2, 1, 0, 3))
    wht = np.ascontiguousarray(
        Wh.T.reshape(DC, 128, HC, 128).transpose(2, 1, 0, 3))

    in_maps = []
    for c in range(NCORES):
        xl = x[c * BL:(c + 1) * BL].reshape(BL * T, D)
        xt = np.ascontiguousarray(xl.T).reshape(DC, 128, BT)
        in_maps.append({
            "xt": xt,
            "wzt": wzt,
            "wht": wht,
            "bz": bz,
            "bh": bh,
            "invtau": np.ascontiguousarray(
                invtau[c * BL:(c + 1) * BL]).reshape(BL, 1, T),
        })
    return in_maps


def _assemble(results):
    outs = []
    for c in range(NCORES):
        o = results[c]["out"]  # [BL, HC, 128, T] bf16
        o = np.transpose(o.astype(np.float32), (0, 3, 1, 2)).reshape(BL, T, H)
        outs.append(o)
    return np.ascontiguousarray(np.concatenate(outs, axis=0))


def _run(inputs, trace=False):
    from concourse.bass_utils import run_bass_kernel_spmd

    bza = np.asarray(inputs["bz"], dtype=np.float32).reshape(-1)
    bha = np.asarray(inputs["bh"], dtype=np.float32).reshape(-1)
    bz0 = float(bza[0]) if np.all(bza == bza[0]) else None
    bh0 = float(bha[0]) if np.all(bha == bha[0]) else None
    key = ("nc", bz0, bh0)
    if key not in _CACHE:
        _CACHE[key] = _build_nc(bz0, bh0)
    nc = _CACHE[key]
    in_maps = _prep_inputs(**inputs)
    res = run_bass_kernel_spmd(nc, in_maps, list(range(NCORES)), trace=trace)
    return _assemble(res.results), res


def kernel(**inputs):
    out, _ = _run(inputs, trace=False)
    return out
